# revision 20
# baseline (speedup 1.0000x reference)
"""ChannelAttentionModule kernel for TRN2 (Bass/Tile), 8-core SPMD.

Computes sigmoid(mean_{hw}(x) @ W.T + b) for x:[16,128,256,256].

Sharding: data-parallel over batch, 2 images per core (64 MiB/core), no
collectives; host concatenates the per-core [2] outputs into [16,1,1,1].

Per-core dataflow (memory-bound; HBM read of the shard is the roofline):
- The shard is read batch-major as one sequential 64 MiB scan of 2 MiB
  *address-contiguous* slabs [128, 4096] (partition p <- off + p*4096).
  Channels then span partition groups, so the host precomputes expanded
  per-slab weights wexp[p, c] = W[channel(p, c)]/HW (1/HW is exact).
- DUAL-RING streaming: even slabs go on the scalar HWDGE ring and are
  consumed by ACT (activation Copy with accum_out -> pact[:, i]); odd
  slabs go on the SP (sync) ring and are consumed by DVE (reduce_sum
  -> pdve[:, i]). 4 buffers per ring (8 total in flight — never exceed
  the 8-deep HWDGE completion credit; bufs > 8 on one ring makes every
  dispatch block on the DMA 8 back, +25 us/core). Each ring carries
  half the bandwidth, so per-ring arrivals are ~9.8 us while consumers
  cost ~4.0-4.6 us (ACT) / ~4.4+2.4 us cross-engine (DVE): both loops
  keep a positive recovery margin even on cores where the cross-engine
  semaphore hop inflates to ~2.4 us. When one ring's pipeline hiccups,
  the 16 SDMA engines drain the other ring's queued packets instead of
  idling, so HBM request occupancy — the currency of the stack's
  arbitration between the two cores sharing it — is retained.
- The scalar ring is self-paced: the program interleaves act(i-4);
  dma(i) so the slot a DMA reuses was freed by the same engine one
  instruction earlier (no cross-engine semaphore in its issue loop).
- The last slab of the scan is split into 4 sub-slabs with dedicated
  buffers: their dma_starts are emitted interleaved before the last 4
  scalar-ring acts (each sub's HWDGE lane credit fires before the
  neighboring act's own data arrives, so the in-order sequencer never
  stalls on them even when the stream runs late), and ring FIFO order
  still lands their data at stream end, where DVE reduces them.
- Tail: DVE applies wexp to the three accumulators (tensor_mul) and
  row-sums them per batch; one [128,2] matmul against a ones column
  contracts partitions; ACT sigmoid(+bias); 8-byte DMA out.
"""

import numpy as np

_B, _C, _HW = 16, 128, 65536  # batch, channels, H*W
_NCORES = 8
_BPC = _B // _NCORES  # batches per core = 2
_NCH = 16  # 2 MiB chunks per batch (b1's last one split finer)
_F = _HW // _NCH  # free-dim elements per full chunk (4096)
_FSUB = _F // 4  # sub-chunk free elems (1024)
_NFULL = 2 * _NCH - 1  # 31 full chunks (b0: 16, b1: 15)
_NSUB = 4

_cached_nc = None


def _chunk_lists():
    """(full, sub) chunk lists.

    full: (bi, flat_offset) for the 31 full [128, 4096] slabs,
    batch-major so the DMA scan is one sequential 64 MiB read. Chunk c
    rides ring c%2 (0 = scalar/ACT, 1 = sync/DVE) with per-ring column
    c//2; b0 chunks form a prefix of each ring's columns.
    sub: flat_offset for the 4 trailing [128, 1024] sub-slabs of
    batch 1's last slab.
    """
    total = _C * _HW
    slab = total // _NCH  # elements per full slab
    full = []
    for s in range(_NCH):
        full.append((0, s * slab))
    for s in range(_NCH - 1):
        full.append((1, s * slab))
    sub = []
    for k in range(_NSUB):
        sub.append((_NCH - 1) * slab + k * (slab // 4))
    return full, sub


_FULL, _SUB = _chunk_lists()
_NA = (_NFULL + 1) // 2  # scalar-ring chunks (16)
_NB = _NFULL // 2  # sync-ring chunks (15)


def _build_nc(bufs=4, asserts=True):
    import concourse.bacc as bacc
    import concourse.tile as tile
    from concourse import mybir

    f32 = mybir.dt.float32
    nc = bacc.Bacc(
        "TRN2",
        target_bir_lowering=False,
        debug=False,
        num_devices=_NCORES,
        enable_asserts=asserts,
    )

    na, nb, nsub = _NA, _NB, _NSUB
    naux = na + nb + nsub + 1  # wexpA, wexpB, wexp_sub, ones

    x = nc.dram_tensor("x", [_BPC, _C * _HW], f32, kind="ExternalInput")
    aux = nc.dram_tensor("wexp", [128, naux], f32, kind="ExternalInput")
    bvec = nc.dram_tensor("bias", [1, 1], f32, kind="ExternalInput")
    out = nc.dram_tensor("out", [1, _BPC], f32, kind="ExternalOutput")

    with tile.TileContext(nc) as tc:
        with (
            tc.tile_pool(name="bigA", bufs=bufs) as bigA,
            tc.tile_pool(name="bigB", bufs=bufs) as bigB,
            tc.tile_pool(name="sub", bufs=nsub) as subp,
            tc.tile_pool(name="small", bufs=1) as small,
            tc.tile_pool(name="psum", bufs=1, space="PSUM") as psum,
        ):
            # Tiny loads via SWDGE (gpsimd), off both HWDGE rings.
            w_sb = small.tile([128, naux], f32)
            nc.gpsimd.dma_start(out=w_sb[:], in_=aux[:])
            b_sb = small.tile([1, 1], f32)
            nc.gpsimd.dma_start(out=b_sb[:], in_=bvec[:])

            pact = small.tile([128, na], f32)  # ACT-owned partials
            pdvf = small.tile([128, nb], f32)  # DVE full-chunk partials
            pdvs = small.tile([128, nsub], f32)  # DVE sub partials

            def dma_chunk(eng, t, bi, off, f):
                eng.dma_start(
                    out=t[:],
                    in_=x[bi, off : off + 128 * f].rearrange(
                        "(p f) -> p f", f=f
                    ),
                )

            def act_consume(i, t):
                nc.scalar.activation(
                    out=t[:],
                    in_=t[:],
                    func=mybir.ActivationFunctionType.Copy,
                    accum_out=pact[:, i : i + 1],
                )

            # Ring A (scalar, even chunks): self-paced act(i-4); dma(i).
            # Ring B (sync, odd chunks): SP issues in order; its slot
            # WAR waits resolve via DVE's reduce of the chunk 4 back.
            a_tiles = []
            for c, (bi, off) in enumerate(_FULL):
                if c % 2 == 0:
                    i = c // 2
                    t = bigA.tile([128, _F], f32, tag="xa")
                    if i >= bufs:
                        act_consume(i - bufs, a_tiles[i - bufs])
                        a_tiles[i - bufs] = None
                    dma_chunk(nc.scalar, t, bi, off, _F)
                    a_tiles.append(t)
                else:
                    i = c // 2
                    t = bigB.tile([128, _F], f32, tag="xb")
                    dma_chunk(nc.sync, t, bi, off, _F)
                    nc.vector.reduce_sum(
                        out=pdvf[:, i : i + 1],
                        in_=t[:],
                        axis=mybir.AxisListType.X,
                    )

            # Remaining ring-A consumers, with the sub-slab DMAs
            # interleaved before the last nsub acts (their lane credits
            # fire before the neighboring act's data, so no stall).
            subtiles = []

            def emit_sub(k):
                st = subp.tile([128, _FSUB], f32, tag="subtile")
                dma_chunk(nc.scalar, st, 1, _SUB[k], _FSUB)
                subtiles.append(st)

            rem = list(range(max(0, na - bufs), na))
            for idx, i in enumerate(rem):
                j = idx - (len(rem) - nsub)
                if 0 <= j < nsub:
                    emit_sub(j)
                act_consume(i, a_tiles[i])
            for k in range(len(subtiles), nsub):
                emit_sub(k)

            for k in range(nsub):
                nc.vector.reduce_sum(
                    out=pdvs[:, k : k + 1],
                    in_=subtiles[k][:],
                    axis=mybir.AxisListType.X,
                )

            # Tail contraction: apply wexp, row-sum per batch (b0 is a
            # prefix of each ring's columns), contract partitions.
            na0 = _NCH - _NCH // 2  # b0 chunks on ring A (8)
            nb0 = _NCH // 2  # b0 chunks on ring B (8)
            wA = small.tile([128, na], f32)
            wB = small.tile([128, nb], f32)
            wS = small.tile([128, nsub], f32)
            rA1 = small.tile([128, 1], f32)
            rB0 = small.tile([128, 1], f32)
            rB1 = small.tile([128, 1], f32)
            rS = small.tile([128, 1], f32)
            acc = small.tile([128, _BPC], f32)
            nc.vector.tensor_mul(wA[:], pact[:], w_sb[:, 0:na])
            nc.vector.tensor_mul(wB[:], pdvf[:], w_sb[:, na : na + nb])
            nc.vector.tensor_mul(
                wS[:], pdvs[:], w_sb[:, na + nb : na + nb + nsub]
            )
            # acc[:,0] = sum(b0 cols of A) + sum(b0 cols of B)
            nc.vector.reduce_sum(
                out=acc[:, 0:1], in_=wA[:, 0:na0], axis=mybir.AxisListType.X
            )
            nc.vector.reduce_sum(
                out=rB0[:], in_=wB[:, 0:nb0], axis=mybir.AxisListType.X
            )
            nc.vector.tensor_add(acc[:, 0:1], acc[:, 0:1], rB0[:])
            # acc[:,1] = sum(b1 cols of A) + sum(b1 cols of B) + subs
            nc.vector.reduce_sum(
                out=rA1[:], in_=wA[:, na0:na], axis=mybir.AxisListType.X
            )
            nc.vector.reduce_sum(
                out=rB1[:], in_=wB[:, nb0:nb], axis=mybir.AxisListType.X
            )
            nc.vector.reduce_sum(
                out=rS[:], in_=wS[:], axis=mybir.AxisListType.X
            )
            nc.vector.tensor_add(rA1[:], rA1[:], rB1[:])
            nc.vector.tensor_add(acc[:, 1:2], rA1[:], rS[:])

            ps = psum.tile([1, _BPC], f32)
            nc.tensor.matmul(
                ps[:],
                w_sb[:, na + nb + nsub : naux],
                acc[:],
                start=True,
                stop=True,
            )

            # sigmoid(att + bias); mean scale already folded into wexp
            res = small.tile([1, _BPC], f32)
            nc.scalar.activation(
                out=res[:],
                in_=ps[:],
                func=mybir.ActivationFunctionType.Sigmoid,
                bias=b_sb[:],
                scale=1.0,
            )
            nc.sync.dma_start(out=out[:], in_=res[:])

    nc.compile()
    return nc


def _prepare_in_maps(x, W, b):
    xs = np.ascontiguousarray(x, dtype=np.float32).reshape(_B, _C * _HW)
    b_col = np.ascontiguousarray(b, dtype=np.float32).reshape(1, 1)
    # wexp[p, c] = W[channel of partition p in chunk c] / HW, where the
    # channel of partition p in chunk (off, f) is (off + p*f) // _HW.
    w_flat = np.asarray(W, dtype=np.float32).reshape(_C)
    p = np.arange(128)[:, None]
    scale = np.float32(1.0 / _HW)
    offs = [off for (_bi, off) in _FULL]
    offA = np.array(offs[0::2])[None, :]  # ring A (even chunks)
    offB = np.array(offs[1::2])[None, :]  # ring B (odd chunks)
    chA = (offA + p * _F) // _HW
    chB = (offB + p * _F) // _HW
    offS = np.array(_SUB)[None, :]
    chS = (offS + p * _FSUB) // _HW
    ones = np.ones((128, 1), dtype=np.float32)
    aux = np.ascontiguousarray(
        np.concatenate(
            [
                w_flat[chA] * scale,
                w_flat[chB] * scale,
                w_flat[chS] * scale,
                ones,
            ],
            axis=1,
        ).astype(np.float32)
    )
    return [
        {
            "x": np.ascontiguousarray(xs[i * _BPC : (i + 1) * _BPC]),
            "wexp": aux,
            "bias": b_col,
        }
        for i in range(_NCORES)
    ]


def _gather(results):
    outs = [np.asarray(results[i]["out"]).reshape(_BPC) for i in range(_NCORES)]
    return np.concatenate(outs, axis=0).reshape(_B, 1, 1, 1).astype(np.float32)


def kernel(x, W, b):
    from concourse.bass_utils import run_bass_kernel_spmd

    global _cached_nc
    if _cached_nc is None:
        _cached_nc = _build_nc()
    in_maps = _prepare_in_maps(x, W, b)
    res = run_bass_kernel_spmd(_cached_nc, in_maps, list(range(_NCORES)))
    return _gather(res.results)


# revision 25
# speedup vs baseline: 1.0123x; 1.0123x over previous
"""ChannelAttentionModule kernel for TRN2 (Bass/Tile), 8-core SPMD.

Computes sigmoid(mean_{hw}(x) @ W.T + b) for x:[16,128,256,256].

Sharding: data-parallel over batch, 2 images per core (64 MiB/core), no
collectives; host concatenates the per-core [2] outputs into [16,1,1,1].

Per-core dataflow (memory-bound; HBM read of the shard is the roofline):
- The shard is read batch-major as one sequential 64 MiB scan of 2 MiB
  *address-contiguous* slabs [128, 4096] (partition p <- off + p*4096).
  Channels then span partition groups, so the host precomputes expanded
  per-slab weights wexp[p, c] = W[channel(p, c)]/HW (1/HW is exact).
- Single-engine streaming loop: the SCALAR engine both issues every
  full-slab DMA (HWDGE) and consumes every full slab (activation Copy
  with accum_out -> pact[:, c]). The program explicitly interleaves
  act(c-D); dma(c), so the slot a DMA reuses was provably freed by the
  same engine one instruction earlier: the steady-state issue loop
  contains NO cross-engine semaphore. Consumer+issue (~4.6 us) < slab
  arrival period (~4.9 us at the 430 GB/s line rate), so the pipeline
  has a deterministic recovery margin after any hiccup.
  (Earlier designs paced DMA issue via a cross-engine slot release —
  DVE/ACT consumer -> SP issuer. On cores where that semaphore hop
  costs ~2.4 us instead of ~0.7 us the loop margin goes negative and
  the stream locks into a ~310 GB/s convoy, +40 us on the shard.)
- The last slab of the scan is split into 4 sub-slabs with dedicated
  buffers (no slot reuse): their dma_starts are emitted interleaved
  before the last 4 tail acts (each sub's HWDGE lane credit — the
  completion of the ring DMA 8 back — provably fires before the
  neighboring act's own data arrives, so the in-order sequencer never
  stalls on them, even when the stream runs late), and ring FIFO order
  still lands their data at stream end, where DVE (otherwise idle)
  reduces them as they land. Keep in-flight DMAs per ring <= 8: the
  HWDGE completion-credit depth is 8, so bufs > 8 makes every dma
  dispatch block on the completion of the DMA 8 back (+25 us/core).
- Tail: DVE applies wexp to both accumulators (tensor_mul) and
  row-sums them; one [128,2] matmul against a ones column contracts
  partitions; ACT sigmoid(+bias); 8-byte DMA out on the idle SP ring.
"""

import numpy as np

_B, _C, _HW = 16, 128, 65536  # batch, channels, H*W
_NCORES = 8
_BPC = _B // _NCORES  # batches per core = 2
_NCH = 8  # 4 MiB chunks per batch (b1's last one split finer)
_F = _HW // _NCH  # free-dim elements per full chunk (8192)
_FSUB = 1024  # sub-chunk free elems
_NFULL = 2 * _NCH - 1  # 15 full chunks (b0: 8, b1: 7)
_NSUB = _F // _FSUB  # 8 sub-slabs for b1's last slab

_cached_nc = None


def _chunk_lists():
    """(full, sub) chunk lists.

    full: (bi, flat_offset, col) for the 31 full [128, 4096] slabs,
    batch-major so the DMA scan is one sequential 64 MiB read.
    sub: (flat_offset, col) for the 4 trailing [128, 1024] sub-slabs of
    batch 1's last slab.
    """
    total = _C * _HW
    slab = total // _NCH  # elements per full slab
    full = []
    for s in range(_NCH):
        full.append((0, s * slab, s))
    for s in range(_NCH - 1):
        full.append((1, s * slab, _NCH + s))
    sub = []
    for k in range(_NSUB):
        sub.append(((_NCH - 1) * slab + k * 128 * _FSUB, k))
    return full, sub


_FULL, _SUB = _chunk_lists()


def _build_nc(bufs=5, asserts=True):
    import concourse.bacc as bacc
    import concourse.tile as tile
    from concourse import mybir

    f32 = mybir.dt.float32
    nc = bacc.Bacc(
        "TRN2",
        target_bir_lowering=False,
        debug=False,
        num_devices=_NCORES,
        enable_asserts=asserts,
    )

    nfull, nsub = len(_FULL), len(_SUB)
    naux = nfull + nsub + 1  # wexp_full, wexp_sub, ones

    x = nc.dram_tensor("x", [_BPC, _C * _HW], f32, kind="ExternalInput")
    aux = nc.dram_tensor("wexp", [128, naux], f32, kind="ExternalInput")
    bvec = nc.dram_tensor("bias", [1, 1], f32, kind="ExternalInput")
    out = nc.dram_tensor("out", [1, _BPC], f32, kind="ExternalOutput")

    with tile.TileContext(nc) as tc:
        with (
            tc.tile_pool(name="big", bufs=bufs) as big,
            tc.tile_pool(name="sub", bufs=nsub) as subp,
            tc.tile_pool(name="small", bufs=1) as small,
            tc.tile_pool(name="psum", bufs=1, space="PSUM") as psum,
        ):
            # Tiny loads via SWDGE (gpsimd) so the scalar HWDGE ring
            # starts streaming x immediately.
            w_sb = small.tile([128, naux], f32)
            nc.gpsimd.dma_start(out=w_sb[:], in_=aux[:])
            b_sb = small.tile([1, 1], f32)
            nc.gpsimd.dma_start(out=b_sb[:], in_=bvec[:])

            pact = small.tile([128, nfull], f32)  # ACT-owned partials
            pdve = small.tile([128, nsub], f32)  # DVE-owned partials

            def act_consume(c, t):
                nc.scalar.activation(
                    out=t[:],
                    in_=t[:],
                    func=mybir.ActivationFunctionType.Copy,
                    accum_out=pact[:, c : c + 1],
                )

            tiles = []
            for c, (bi, off, _col) in enumerate(_FULL):
                t = big.tile([128, _F], f32, tag="xtile")
                # Same-engine pipeline: free the slot this DMA reuses
                # (consumer of chunk c-bufs) right before issuing, so
                # the issue never blocks on a cross-engine semaphore.
                if c >= bufs:
                    act_consume(c - bufs, tiles[c - bufs])
                    tiles[c - bufs] = None
                nc.scalar.dma_start(
                    out=t[:],
                    in_=x[bi, off : off + 128 * _F].rearrange(
                        "(p f) -> p f", f=_F
                    ),
                )
                tiles.append(t)

            # Remaining full-slab consumers (arrival-paced), with the
            # sub-slab DMAs interleaved before the last nsub acts.
            # Emitting sub dma_starts BEFORE these acts would stall the
            # in-order scalar sequencer on convoyed cores: each sub
            # dispatch is HWDGE-lane-credit-gated on the completion of
            # the ring DMA 8 back, so a late stream pushes ALL the
            # remaining acts past stream end (~13 us serial pileup).
            # Interleaved, sub k's credit (full chunk completion) fires
            # before act(c)'s own data dependency, so no added stall,
            # and ring FIFO order still lands the subs at stream end.
            subtiles = []

            def emit_sub(k):
                off, _col = _SUB[k]
                st = subp.tile([128, _FSUB], f32, tag="subtile")
                nc.scalar.dma_start(
                    out=st[:],
                    in_=x[1, off : off + 128 * _FSUB].rearrange(
                        "(p f) -> p f", f=_FSUB
                    ),
                )
                subtiles.append(st)

            rem = list(range(max(0, nfull - bufs), nfull))
            # When nsub > len(rem), lead with the extras (their lane
            # credits are mid-stream completions, ready by now), then
            # attach the rest 1:1 before the last acts.
            lead = max(0, nsub - len(rem))
            for k in range(lead):
                emit_sub(k)
            for i, c in enumerate(rem):
                j = lead + i - (len(rem) - (nsub - lead))
                if lead <= j < nsub:
                    emit_sub(j)
                act_consume(c, tiles[c])
            for k in range(len(subtiles), nsub):
                emit_sub(k)

            for k in range(nsub):
                nc.vector.reduce_sum(
                    out=pdve[:, k : k + 1],
                    in_=subtiles[k][:],
                    axis=mybir.AxisListType.X,
                )

            # Tail contraction: apply wexp, row-sum per batch, contract
            # partitions with one matmul.
            wfull = small.tile([128, nfull], f32)
            wsub = small.tile([128, nsub], f32)
            r1 = small.tile([128, 1], f32)
            r2 = small.tile([128, 1], f32)
            acc = small.tile([128, _BPC], f32)
            nc.vector.tensor_mul(wfull[:], pact[:], w_sb[:, 0:nfull])
            nc.vector.tensor_mul(
                wsub[:], pdve[:], w_sb[:, nfull : nfull + nsub]
            )
            nc.vector.reduce_sum(
                out=acc[:, 0:1],
                in_=wfull[:, 0:_NCH],
                axis=mybir.AxisListType.X,
            )
            nc.vector.reduce_sum(
                out=r1[:], in_=wfull[:, _NCH:nfull], axis=mybir.AxisListType.X
            )
            nc.vector.reduce_sum(
                out=r2[:], in_=wsub[:], axis=mybir.AxisListType.X
            )
            nc.vector.tensor_add(acc[:, 1:2], r1[:], r2[:])

            ps = psum.tile([1, _BPC], f32)
            nc.tensor.matmul(
                ps[:],
                w_sb[:, nfull + nsub : naux],
                acc[:],
                start=True,
                stop=True,
            )

            # sigmoid(att + bias); mean scale already folded into wexp
            res = small.tile([1, _BPC], f32)
            nc.scalar.activation(
                out=res[:],
                in_=ps[:],
                func=mybir.ActivationFunctionType.Sigmoid,
                bias=b_sb[:],
                scale=1.0,
            )
            nc.sync.dma_start(out=out[:], in_=res[:])

    nc.compile()
    return nc


def _prepare_in_maps(x, W, b):
    xs = np.ascontiguousarray(x, dtype=np.float32).reshape(_B, _C * _HW)
    b_col = np.ascontiguousarray(b, dtype=np.float32).reshape(1, 1)
    # wexp[p, c] = W[channel of partition p in chunk c] / HW, where the
    # channel of partition p in chunk (off, f) is (off + p*f) // _HW.
    w_flat = np.asarray(W, dtype=np.float32).reshape(_C)
    p = np.arange(128)[:, None]
    off_f = np.array([off for (_bi, off, _c) in _FULL])[None, :]
    ch_f = (off_f + p * _F) // _HW
    off_s = np.array([off for (off, _c) in _SUB])[None, :]
    ch_s = (off_s + p * _FSUB) // _HW
    scale = np.float32(1.0 / _HW)
    ones = np.ones((128, 1), dtype=np.float32)
    aux = np.ascontiguousarray(
        np.concatenate(
            [w_flat[ch_f] * scale, w_flat[ch_s] * scale, ones], axis=1
        ).astype(np.float32)
    )
    return [
        {
            "x": np.ascontiguousarray(xs[i * _BPC : (i + 1) * _BPC]),
            "wexp": aux,
            "bias": b_col,
        }
        for i in range(_NCORES)
    ]


def _gather(results):
    outs = [np.asarray(results[i]["out"]).reshape(_BPC) for i in range(_NCORES)]
    return np.concatenate(outs, axis=0).reshape(_B, 1, 1, 1).astype(np.float32)


def kernel(x, W, b):
    from concourse.bass_utils import run_bass_kernel_spmd

    global _cached_nc
    if _cached_nc is None:
        _cached_nc = _build_nc()
    in_maps = _prepare_in_maps(x, W, b)
    res = run_bass_kernel_spmd(_cached_nc, in_maps, list(range(_NCORES)))
    return _gather(res.results)
